# revision 8
# baseline (speedup 1.0000x reference)
"""CNN+LSTM seq2seq kernel for trn2, 8-core data parallel.

Model (per core, batch 64):
  conv1d(16->64, k=5, same) -> relu -> maxpool2 -> LSTM(64->512) over 512 steps
  -> autoregressive LSTM(1->512) decoder 64 steps with linear head(512->1).

Sharding: batch 512 split across 8 cores; weights replicated. No collectives.
"""

import numpy as np

import concourse.bass as bass
import concourse.mybir as mybir
import concourse.tile as tile_mod
from concourse import bacc
from concourse.bass import ds, ts
from concourse.masks import make_identity

F32 = mybir.dt.float32
AF = mybir.ActivationFunctionType

B = 64        # batch per core
S = 1024      # input seq len
CIN = 16
OC = 64       # conv out channels
KW = 5
T2 = 512      # encoder steps after pool
H = 512       # hidden
G = 4 * H     # gates
OUT_STEPS = 64
NCORES = 8

# gate block permutation: torch order [i f g o] (128-row blocks 0..15)
# -> our layout [i f o g] so sigmoid covers [0:1536), tanh(g) covers [1536:2048)
PERM = [0, 1, 2, 3, 4, 5, 6, 7, 12, 13, 14, 15, 8, 9, 10, 11]


def build_nc(unroll_note=""):
    nc = bacc.Bacc(None, target_bir_lowering=False, debug=False)

    # ---------- DRAM I/O ----------
    x_d = nc.dram_tensor("x", [B, S, CIN], F32, kind="ExternalInput")
    dstart_d = nc.dram_tensor("decoder_start", [B, 1], F32, kind="ExternalInput")
    convw_d = nc.dram_tensor("conv_w", [OC, CIN, KW], F32, kind="ExternalInput")
    convb_d = nc.dram_tensor("conv_b", [OC], F32, kind="ExternalInput")
    encWih_d = nc.dram_tensor("enc_Wih", [G, OC], F32, kind="ExternalInput")
    encWhh_d = nc.dram_tensor("enc_Whh", [G, H], F32, kind="ExternalInput")
    encb_d = nc.dram_tensor("enc_b", [G], F32, kind="ExternalInput")
    decWih_d = nc.dram_tensor("dec_Wih", [G, 1], F32, kind="ExternalInput")
    decWhh_d = nc.dram_tensor("dec_Whh", [G, H], F32, kind="ExternalInput")
    decb_d = nc.dram_tensor("dec_b", [G], F32, kind="ExternalInput")
    headw_d = nc.dram_tensor("head_w", [1, H], F32, kind="ExternalInput")
    headb_d = nc.dram_tensor("head_b", [1], F32, kind="ExternalInput")
    out_d = nc.dram_tensor("out", [B, OUT_STEPS], F32, kind="ExternalOutput")

    with tile_mod.TileContext(nc) as tc:
        with tc.tile_pool(name="dram", bufs=1, space="DRAM") as dramp:
            # encoder inputs staged in DRAM as [t', b, oc] so the per-step
            # stationary load is one contiguous 16KB block
            enc_x = dramp.tile([T2, B, OC], F32)

            with tc.tile_pool(name="const", bufs=1) as cn:
                identity = cn.tile([128, 128], F32)
                make_identity(nc, identity)
                id64 = identity[:64, :64]

                # persistent weights (stream operands)
                hW = [cn.tile([128, G], F32, name=f"hW{k}") for k in range(4)]
                xW = cn.tile([OC + 1, G], F32)          # rows 0..63 Wih.T, row 64 enc_b
                dhW = [cn.tile([128, G], F32, name=f"dhW{k}") for k in range(4)]
                dxW = cn.tile([2, G], F32)              # row0 dec_Wih.T, row1 dec_b
                cwT = [cn.tile([CIN, OC], F32, name=f"cwT{k}") for k in range(KW)]
                cb = cn.tile([OC, 1], F32)
                hdT = cn.tile([128, 4], F32)            # head_w.T chunks as columns
                hb = cn.tile([1, 1], F32)
                ones_row = cn.tile([1, B], F32)
                nc.vector.memset(ones_row, 1.0)

                # persistent state
                c_st = cn.tile([B, H], F32)
                hT = cn.tile([128, 4 * B], F32)         # h.T, chunk k at [:, 64k:64k+64]
                sig = cn.tile([B, 3 * H], F32)          # sigmoid(i,f,o)
                tg = cn.tile([B, H], F32)
                tcell = cn.tile([B, H], F32)
                h_st = cn.tile([B, H], F32)
                t1 = cn.tile([B, H], F32)
                t2 = cn.tile([B, H], F32)
                outF = cn.tile([B, OUT_STEPS], F32)
                nc.vector.memset(c_st, 0.0)
                nc.vector.memset(hT, 0.0)

                # ---------- weight prep (on-chip transposes) ----------
                with (
                    tc.tile_pool(name="wtmp", bufs=3) as wt,
                    tc.tile_pool(name="wps", bufs=3, space="PSUM") as wps,
                ):
                    def prep_whh(src_d, dst_tiles):
                        for jb in range(16):
                            n = PERM.index(jb)
                            wtmp = wt.tile([128, H], F32, tag="wtmp")
                            nc.sync.dma_start(out=wtmp, in_=src_d[128 * jb:128 * (jb + 1), :])
                            for kc in range(4):
                                wtp = wps.tile([128, 128], F32, tag="wtp")
                                nc.tensor.transpose(wtp, wtmp[:, 128 * kc:128 * (kc + 1)], identity)
                                eng = nc.scalar if (kc % 2 == 0) else nc.vector
                                if eng is nc.scalar:
                                    nc.scalar.copy(dst_tiles[kc][:, 128 * n:128 * (n + 1)], wtp)
                                else:
                                    nc.vector.tensor_copy(dst_tiles[kc][:, 128 * n:128 * (n + 1)], wtp)

                    prep_whh(encWhh_d, hW)
                    prep_whh(decWhh_d, dhW)

                    # enc_Wih.T into xW rows 0..63
                    for jb in range(16):
                        n = PERM.index(jb)
                        wtmp2 = wt.tile([128, OC], F32, tag="wtmp2")
                        nc.sync.dma_start(out=wtmp2, in_=encWih_d[128 * jb:128 * (jb + 1), :])
                        wtp = wps.tile([128, 128], F32, tag="wtp")
                        nc.tensor.transpose(wtp[:OC, :128], wtmp2, identity)
                        nc.scalar.copy(xW[0:OC, 128 * n:128 * (n + 1)], wtp[:OC, :128])
                    # biases / vectors (tiny strided DMAs)
                    for n in range(16):
                        jb = PERM[n]
                        nc.sync.dma_start(out=xW[OC:OC + 1, 128 * n:128 * (n + 1)],
                                          in_=encb_d[None, 128 * jb:128 * (jb + 1)])
                        nc.sync.dma_start(out=dxW[0:1, 128 * n:128 * (n + 1)],
                                          in_=decWih_d[128 * jb:128 * (jb + 1), :].rearrange("a b -> b a"))
                        nc.sync.dma_start(out=dxW[1:2, 128 * n:128 * (n + 1)],
                                          in_=decb_d[None, 128 * jb:128 * (jb + 1)])
                    # conv weights: cwT[k][ic, oc] = conv_w[oc, ic, k]
                    for k in range(KW):
                        nc.sync.dma_start(
                            out=cwT[k],
                            in_=convw_d[:, :, k].rearrange("oc ic -> ic oc"),
                        )
                    nc.sync.dma_start(out=cb, in_=convb_d[:, None])
                    # head_w.T chunks as columns of hdT
                    for kc in range(4):
                        nc.sync.dma_start(
                            out=hdT[:, kc:kc + 1],
                            in_=headw_d[:, 128 * kc:128 * (kc + 1)].rearrange("a b -> b a"),
                        )
                    nc.sync.dma_start(out=hb, in_=headb_d[:, None])

                # ---------- conv + pool -> enc_x ----------
                with (
                    tc.tile_pool(name="conv", bufs=2) as cp,
                    tc.tile_pool(name="convps", bufs=2, space="PSUM") as cpp,
                ):
                    for b in range(B):
                        # xTb rows 0:16 hold x[b].T with 2-col zero pads; rows
                        # 16:32 are scratch written by the 32-partition unpack
                        # copies (PSUM reads must start 32-aligned).
                        xTb = cp.tile([32, S + 4 + 4], F32, tag="xTb")
                        nc.vector.memset(xTb[0:CIN, 0:2], 0.0)
                        nc.vector.memset(xTb[0:CIN, 2 + S:2 + S + 2], 0.0)
                        for half in range(2):
                            xb_raw = cp.tile([128, 128], F32, tag="xb_raw", bufs=3)
                            nc.sync.dma_start(
                                out=xb_raw.rearrange("p (a c) -> p a c", c=32)[:, :, 0:CIN],
                                in_=x_d[b].rearrange("(a p) c -> p a c", p=128)[:, 4 * half:4 * half + 4, :],
                            )
                            xtp = cpp.tile([128, 128], F32, tag="xtp")
                            nc.tensor.transpose(xtp, xb_raw, identity)
                            for a in range(4):
                                blk = xtp[32 * a:32 * (a + 1), :]
                                dst = xTb[:, 2 + 128 * (4 * half + a):2 + 128 * (4 * half + a + 1)]
                                if a % 2 == 0:
                                    nc.scalar.copy(dst, blk)
                                else:
                                    nc.vector.tensor_copy(dst, blk)
                        yb = cp.tile([OC, S], F32, tag="yb")
                        for half in range(2):
                            cps = cpp.tile([OC, 512], F32, tag="cps")
                            for k in range(KW):
                                nc.tensor.matmul(
                                    cps,
                                    lhsT=cwT[k],
                                    rhs=xTb[0:CIN, k + 512 * half:k + 512 * half + 512],
                                    start=(k == 0),
                                    stop=(k == KW - 1),
                                )
                            nc.scalar.activation(yb[:, 512 * half:512 * (half + 1)], cps,
                                                 AF.Relu, bias=cb[:, 0:1])
                        pooled = cp.tile([OC, T2], F32, tag="pooled")
                        yb_pairs = yb.rearrange("p (t two) -> p t two", two=2)
                        nc.vector.tensor_max(pooled, yb_pairs[:, :, 0], yb_pairs[:, :, 1])
                        poolT = cp.tile([128, 4 * OC], F32, tag="poolT")
                        for q in range(4):
                            ptp = cpp.tile([128, OC], F32, tag="ptp")
                            nc.tensor.transpose(ptp, pooled[:, 128 * q:128 * (q + 1)], id64)
                            if q % 2 == 0:
                                nc.scalar.copy(poolT[:, OC * q:OC * (q + 1)], ptp)
                            else:
                                nc.vector.tensor_copy(poolT[:, OC * q:OC * (q + 1)], ptp)
                        for q in range(4):
                            nc.sync.dma_start(
                                out=enc_x[128 * q:128 * (q + 1), b, :],
                                in_=poolT[:, OC * q:OC * (q + 1)],
                            )

                # ---------- encoder + decoder ----------
                with (
                    tc.tile_pool(name="step", bufs=2) as sp,
                    tc.tile_pool(name="lps", bufs=1, space="PSUM") as lp,
                ):
                    gps = lp.tile([B, G], F32, tag="gates")

                    def lstm_elementwise(gps_):
                        """gates psum -> updates c_st, h_st, hT."""
                        nc.scalar.activation(sig, gps_[:, 0:3 * H], AF.Sigmoid)
                        nc.scalar.activation(tg, gps_[:, 3 * H:4 * H], AF.Tanh)
                        nc.vector.tensor_mul(t1, sig[:, H:2 * H], c_st)
                        nc.vector.tensor_mul(t2, sig[:, 0:H], tg)
                        nc.vector.tensor_add(c_st, t1, t2)
                        nc.scalar.activation(tcell, c_st, AF.Tanh)
                        nc.vector.tensor_mul(h_st, sig[:, 2 * H:3 * H], tcell)
                        htp = lp.tile([128, 4 * B], F32, tag="htp", bufs=2)
                        for q in range(4):
                            nc.tensor.transpose(htp[:, B * q:B * (q + 1)],
                                                h_st[:, 128 * q:128 * (q + 1)], id64)
                        nc.scalar.copy(hT, htp)

                    # --- encoder loop (static unroll) ---
                    for t in range(T2):
                        xb = sp.tile([B, OC], F32, tag="xb", bufs=4)
                        nc.sync.dma_start(out=xb, in_=enc_x[t])
                        xps = lp.tile([B, OC], F32, tag="small", bufs=2)
                        nc.tensor.transpose(xps, xb, id64)
                        xsT = sp.tile([OC + 1, B], F32, tag="xsT", bufs=3)
                        nc.sync.dma_start(out=xsT[OC:OC + 1, :], in_=ones_row)
                        nc.vector.tensor_copy(xsT[0:OC, :], xps)
                        for n in range(4):
                            gsl = gps[:, 512 * n:512 * (n + 1)]
                            for kc in range(4):
                                nc.tensor.matmul(gsl, lhsT=hT[:, B * kc:B * (kc + 1)],
                                                 rhs=hW[kc][:, 512 * n:512 * (n + 1)],
                                                 start=(kc == 0), stop=False)
                            nc.tensor.matmul(gsl, lhsT=xsT,
                                             rhs=xW[:, 512 * n:512 * (n + 1)],
                                             start=False, stop=True)
                        lstm_elementwise(gps)

                    # --- decoder prep ---
                    dssb = sp.tile([B, 1], F32, tag="dssb", bufs=1)
                    nc.sync.dma_start(out=dssb, in_=dstart_d[:, :])
                    dsps = lp.tile([1, B], F32, tag="small", bufs=2)
                    nc.tensor.transpose(dsps, dssb, id64)
                    aug = sp.tile([2, B], F32, tag="aug", bufs=3)
                    nc.sync.dma_start(out=aug[1:2, :], in_=ones_row)
                    nc.vector.tensor_copy(aug[0:1, :], dsps)

                    # --- decoder loop ---
                    for t in range(OUT_STEPS):
                        for n in range(4):
                            gsl = gps[:, 512 * n:512 * (n + 1)]
                            for kc in range(4):
                                nc.tensor.matmul(gsl, lhsT=hT[:, B * kc:B * (kc + 1)],
                                                 rhs=dhW[kc][:, 512 * n:512 * (n + 1)],
                                                 start=(kc == 0), stop=False)
                            nc.tensor.matmul(gsl, lhsT=aug,
                                             rhs=dxW[:, 512 * n:512 * (n + 1)],
                                             start=False, stop=True)
                        lstm_elementwise(gps)
                        # head: pred.T = head_w @ h.T + head_b
                        hps = lp.tile([1, B], F32, tag="small", bufs=2)
                        for kc in range(4):
                            nc.tensor.matmul(hps, lhsT=hdT[:, kc:kc + 1],
                                             rhs=hT[:, B * kc:B * (kc + 1)],
                                             start=(kc == 0), stop=False)
                        nc.tensor.matmul(hps, lhsT=hb, rhs=ones_row,
                                         start=False, stop=True)
                        predsb = sp.tile([1, B], F32, tag="predsb", bufs=2)
                        nc.scalar.copy(predsb, hps)
                        opc = lp.tile([B, 1], F32, tag="small", bufs=2)
                        nc.tensor.transpose(opc, predsb, identity[:1, :1])
                        nc.scalar.copy(outF[:, t:t + 1], opc)
                        if t + 1 < OUT_STEPS:
                            aug = sp.tile([2, B], F32, tag="aug", bufs=3)
                            nc.sync.dma_start(out=aug[1:2, :], in_=ones_row)
                            nc.vector.tensor_copy(aug[0:1, :], predsb)

                    nc.sync.dma_start(out=out_d[:, :], in_=outF)

    nc.compile()
    return nc


_CACHED = {}


def kernel(**inputs):
    """Full-input entry: shard batch across 8 cores, run SPMD, gather."""
    from concourse.bass_utils import run_bass_kernel_spmd

    if "nc" not in _CACHED:
        _CACHED["nc"] = build_nc()
    nc = _CACHED["nc"]

    full = {k: np.ascontiguousarray(np.asarray(v, dtype=np.float32)) for k, v in inputs.items()}
    per_core = []
    for c in range(NCORES):
        sl = slice(c * B, (c + 1) * B)
        m = {}
        for k, v in full.items():
            if k in ("x", "decoder_start"):
                m[k] = np.ascontiguousarray(v[sl])
            else:
                m[k] = v
        per_core.append(m)

    res = run_bass_kernel_spmd(nc, per_core, core_ids=list(range(NCORES)))
    outs = [r["out"] for r in res.results]
    return np.concatenate(outs, axis=0)
